# revision 27
# baseline (speedup 1.0000x reference)
"""BP-MLL loss kernel for Trainium2, data-parallel over 8 NeuronCores.

Math: the reference loss is
    L = mean_b  (1/(n_pos_b * n_neg_b)) * sum_{k in Y_b, l in Ybar_b} exp(c_bl - c_bk)
The pairwise sum is separable:
    sum_{k,l} yf_k * ybar_l * exp(c_l) * exp(-c_k)
      = (sum_l ybar_l * exp(c_l)) * (sum_k yf_k * exp(-c_k))  =  S1_b * S2_b
so each batch row only needs two masked exp-sums (O(L) instead of O(L^2)).

Each element contributes to exactly ONE of the two sums (negative
labels to S1 via e^{+c}, positive labels to S2 via e^{-c}), so only L
exponents per row are needed: d = c*(1-2y) (exact sign flip) and
e^{d_j} is that element's term. The device work per batch shard is ONE
exp over L values and a per-partition accumulation, which the ACT
engine does in a single fused instruction.

Sharding: B=32 rows split 4-per-core across 8 cores. Host-side packing
per core builds a [68, 128] f32 tile: each row gets 17 partitions
(ceil(n_neg/128) S1-partitions, then ceil(n_pos/128) S2-partitions;
gaps filled with -1000, whose exp underflows to exactly 0, so
partition sums never mix groups and filler adds 0). One DMA in, one
activation(Exp, accum_out) producing a [68, 1] column of partial sums,
one DMA out. The host splits each row's partials at the group boundary
(recomputed from y), applies 1/(n_pos*n_neg) and the batch mean.

Written in raw Bass (explicit semaphores): the TileContext tail drain
emits a multi-wait Drain instruction that this container's walrus
rejects ("Too many sync wait commands").

Latency-oriented choices (the kernel is ~5 us of fixed DMA/sem latency):
  - everything rides ONE 34 KB input DMA (512 B per partition,
    full-rate descriptors); a second DMA on any queue serializes
    ~0.8-1.5 us through the DGE/DMA pipe stages.
  - the Bass() constructor preamble (4 const memsets + all-engine
    barrier, ~900 ns) is stripped; the only constant needed (a zero
    bias column for the Exp activation) is memset by the otherwise
    idle Pool engine, sem-guarded off the critical path.
  - a throwaway exp on garbage runs on ACT before the input arrives so
    the hardware Exp table load happens under the DMA wait.
"""

import sys
from contextlib import ExitStack

import numpy as np

for _p in ("/opt/trn_rl_repo",):
    if _p not in sys.path:
        sys.path.append(_p)

B, L = 32, 2048
N_CORES = 8
B_SHARD = B // N_CORES  # 4 batch rows per core
JW = 128  # free elements per partition (512 B: full-rate DMA descriptors)
P_ROW = 17  # partitions per row: ceil(n_neg/128) + ceil(n_pos/128) <= 17
P = B_SHARD * P_ROW  # 68 partitions per core
PAD = -1000.0  # exp(-1000) == 0.0 in f32: group-boundary filler

_CACHE = {}


def _strip_preamble(nc):
    """Remove the const-AP memsets and the all-engine barrier that
    bass.Bass() emits at construction (~900 ns on the critical path).
    Nothing in this kernel reads the const APs, and all cross-engine
    ordering is provided by this kernel's own semaphores."""
    bb0 = nc.m.functions[0].blocks[0]
    insts = bb0.instructions
    keep = [i for i in insts if type(i).__name__ in ("InstCall", "InstRegisterMove")]
    while insts:
        insts.pop()
    for i in keep:
        insts.append(i)


def _strip_regmoves(nc):
    """Drop the per-engine register-preset moves (imm 0 / 0xffffffff)
    from the entry block; nothing in this kernel's instruction stream
    reads those registers."""
    bb0 = nc.m.functions[0].blocks[0]
    insts = bb0.instructions
    keep = [i for i in insts if type(i).__name__ == "InstCall"]
    while insts:
        insts.pop()
    for i in keep:
        insts.append(i)


def _strip_end_barrier(nc):
    """Drop the Block-exit all-engine barrier (drain + event-semaphore
    handshake). Each engine halts on its own; the output DMA is already
    guaranteed complete by the explicit out_sem wait on SP."""
    for bb in nc.m.functions[0].blocks:
        if bb.name.endswith("_end"):
            insts = bb.instructions
            while insts:
                insts.pop()


def _strip_end_branches(nc):
    """Drop each engine block's trailing jump into the (now empty) end
    block — the engines simply halt at the end of their own block. The
    SP branch is the last event on the critical path (~50 ns)."""
    for bb in nc.m.functions[0].blocks:
        insts = bb.instructions
        if insts and type(insts[-1]).__name__ == "InstUnconditionalBranch":
            insts.pop()


def _build_bass(final_wait=True):
    import concourse.bass as bass
    from concourse import mybir

    F32 = mybir.dt.float32
    Exp = mybir.ActivationFunctionType.Exp

    nc = bass.Bass()
    _strip_preamble(nc)

    cm_in = nc.declare_dram_parameter("cm", [P, JW], F32, isOutput=False)
    out = nc.declare_dram_parameter("acc", [P, 1], F32, isOutput=True)

    with ExitStack() as es:
        cm_sb = es.enter_context(nc.sbuf_tensor([P, JW], F32))
        e_junk = es.enter_context(nc.sbuf_tensor([P, JW], F32))
        acc = es.enter_context(nc.sbuf_tensor([P, 1], F32))
        bias0 = es.enter_context(nc.sbuf_tensor([P, 1], F32))

        cm_sem = es.enter_context(nc.semaphore("cm_sem"))
        bias_sem = es.enter_context(nc.semaphore("bias_sem"))
        act_sem = es.enter_context(nc.semaphore("act_sem"))
        out_sem = es.enter_context(nc.semaphore("out_sem"))

        block = es.enter_context(nc.Block())

        @block.sync
        def _(sync):
            sync.dma_start(out=cm_sb[:], in_=cm_in[:]).then_inc(cm_sem, 16)
            # act_sem wait embedded in the DMA: skips one SEQ dispatch hop
            sync.dma_start(out=out[:], in_=acc[:]).wait_op(
                act_sem, 1, "sem-ge"
            ).then_inc(out_sem, 16)
            if final_wait:
                sync.wait_ge(out_sem, 16)

        @block.scalar
        def _(scalar):
            # Throwaway exp: forces the hardware Exp table load while the
            # input DMA is still in flight. Reads/writes garbage, never read.
            scalar.activation(
                out=e_junk[:, 0:1], in_=e_junk[:, 0:1], func=Exp, bias=bias0[:]
            )
            scalar.wait_ge(bias_sem, 1)
            # acc[p] = sum_j exp(cm[p, j]); cm_sem wait embedded
            scalar.activation(
                out=e_junk[:], in_=cm_sb[:], func=Exp, bias=bias0[:],
                accum_out=acc[:],
            ).wait_op(cm_sem, 16, "sem-ge").then_inc(act_sem, 1)

        @block.gpsimd
        def _(gpsimd):
            gpsimd.memset(bias0[:], 0.0)
            gpsimd.drain().then_inc(bias_sem, 1)

    _strip_regmoves(nc)
    _strip_end_barrier(nc)
    _strip_end_branches(nc)
    return nc


def _get_nc():
    if "nc" not in _CACHE:
        _CACHE["nc"] = _build_bass()
    return _CACHE["nc"]


def _pack(c, y):
    """Per-core host packing: [4,2048] c + 0/1 y -> [P, 128] f32 of
    exponents d = c*(1-2y) (exact sign flip: e^d is the S1 term e^{+c}
    on negatives, the S2 term e^{-c} on positives). Each row's S1 group
    then S2 group are laid out on partition-aligned blocks with -1000
    filler (exp -> exactly 0), so per-partition accumulator sums never
    mix the two groups."""
    c = np.asarray(c, dtype=np.float32)
    pos = np.asarray(y) == 1
    out = np.full((P, JW), PAD, np.float32)
    for b in range(B_SHARD):
        neg_v = c[b][~pos[b]]  # S1 exponents: +c on negative labels
        pos_v = -c[b][pos[b]]  # S2 exponents: -c on positive labels
        base = b * P_ROW
        q1 = (len(neg_v) + JW - 1) // JW
        q2 = (len(pos_v) + JW - 1) // JW
        out[base : base + q1].reshape(-1)[: len(neg_v)] = neg_v
        out[base + q1 : base + q1 + q2].reshape(-1)[: len(pos_v)] = pos_v
    return out


def _run_device(c, y, trace=False):
    from concourse.bass_utils import run_bass_kernel_spmd

    c = np.asarray(c)
    y = np.asarray(y)
    in_maps = [
        {"cm": _pack(c[i * B_SHARD : (i + 1) * B_SHARD],
                     y[i * B_SHARD : (i + 1) * B_SHARD])}
        for i in range(N_CORES)
    ]
    return run_bass_kernel_spmd(
        _get_nc(), in_maps, core_ids=list(range(N_CORES)), trace=trace
    )


def _combine(results, y):
    """results: per-core dicts with 'acc' [P, 1] f32. y: full [32, 2048]."""
    n_pos = (np.asarray(y) == 1).sum(axis=1)  # [B] ints
    n_neg = L - n_pos
    total = 0.0
    for i, r in enumerate(results):
        acc = r["acc"].astype(np.float64).reshape(B_SHARD, P_ROW)
        for b in range(B_SHARD):
            gb = i * B_SHARD + b
            q1 = (int(n_neg[gb]) + JW - 1) // JW
            s1 = acc[b, :q1].sum()
            s2 = acc[b, q1:].sum()  # filler partitions contribute exactly 0
            total += s1 * s2 / (float(n_pos[gb]) * float(n_neg[gb]))
    return np.float32(total / B)


def kernel(c, y):
    y = np.asarray(y)
    res = _run_device(np.asarray(c), y)
    return np.asarray(_combine(res.results, y), dtype=np.float32)
